# revision 15
# baseline (speedup 1.0000x reference)
"""Trainium2 Bass kernel for nn_Decoder (additive-attention + LSTM decoder).

Reference computation (per batch b, T=128 steps):
    h, c = 0
    enc_proj[b,t,:] = enc[b,t,:] @ W1_enc + b1          (time-invariant, hoisted)
    per step s:
      hc_proj[b,:]  = [h, c] @ W1_hc  (+ b1 folded here)
      scores[b,t]   = tanh(enc_proj[b,t,:] + hc_proj[b,:]) @ w2      (+b2 dropped:
                       softmax-invariant)
      attn          = softmax_t(scores)
      y_tilde[b]    = (sum_t attn * (enc @ fc_w)[b,t]) + y[b,s]*fc_w[E] + fc_b
      gates         = outer(w_ih, y_tilde) + h @ w_hh.T + (b_ih + b_hh)
      LSTM cell update (sigmoid via tanh(x/2) to stay in one ACT table set)
    out[b] = h @ fcf_w[:D] + (sum_t attn * (enc @ fcf_w[D:]))[b] + fcf_b

Device layout: batch sharded 8 ways (64/core).  Feature-on-partition layout:
  enc_projT  [e=128p x2, (t,b) free, t-major]   bf16
  tanh stage [128, 8192] x2                     bf16  (ACT is the bottleneck)
  scores     via w2-stationary matmuls, 4-way col-tiled, M=1
  state h,c  [128p = d%128, 64*blk + b]         f32
"""

import os

import numpy as np
import ml_dtypes

DBG = set(filter(None, os.environ.get("KDBG", "").split(",")))
KSTOP = int(os.environ.get("KSTOP", "99"))  # bisect: emit stages < KSTOP only

B, T, E, D, OUT = 512, 128, 256, 256, 1
NCORES = 8
BL = B // NCORES  # 64 batch per core
NSTEPS = T

F32 = np.float32
BF16 = ml_dtypes.bfloat16

_LAST_RESULTS = None  # stashed BassKernelResults for test.py
_LAST_WALL_NS = None  # wall-clock of the SPMD execute call (timing proxy)


def _host_prepare(inputs):
    """Per-core input dicts: layout transforms only (plus tiny O(B*T*E) matvecs
    for the fc_w / fcf_w contractions of the attention context)."""
    enc = np.asarray(inputs["input_encoded"], F32)        # [B, T, E]
    y_hist = np.asarray(inputs["y_history"], F32)         # [B, T]
    w1 = np.asarray(inputs["attn_w1"], F32)               # [2D+E, E]
    b1 = np.asarray(inputs["attn_b1"], F32)               # [E]
    w2 = np.asarray(inputs["attn_w2"], F32)               # [E, 1]
    w_ih = np.asarray(inputs["lstm_w_ih"], F32)           # [4D, 1]
    w_hh = np.asarray(inputs["lstm_w_hh"], F32)           # [4D, D]
    b_ih = np.asarray(inputs["lstm_b_ih"], F32)           # [4D]
    b_hh = np.asarray(inputs["lstm_b_hh"], F32)           # [4D]
    fc_w = np.asarray(inputs["fc_w"], F32)                # [E+1, 1]
    fc_b = np.asarray(inputs["fc_b"], F32)                # [1]
    fcf_w = np.asarray(inputs["fcf_w"], F32)              # [D+E, 1]
    fcf_b = np.asarray(inputs["fcf_b"], F32)              # [1]

    w1_hc = np.ascontiguousarray(w1[: 2 * D, :])          # [512, 256]
    w1_enc = np.ascontiguousarray(w1[2 * D :, :])         # [256, 256]

    # LSTM: all four gates go through tanh(0.5*x).  sigmoid(x)=(tanh(x/2)+1)/2
    # needs x as-is; tanh(g) needs 2*g pre-scaled.
    gscale = np.ones((4 * D,), F32)
    gscale[2 * D : 3 * D] = 2.0  # g-gate rows
    w_hhT = np.ascontiguousarray((w_hh * gscale[:, None]).T)     # [256, 1024]
    b_row = ((b_ih + b_hh) * gscale).reshape(1, 4 * D)            # [1, 1024]
    w_row = (w_ih[:, 0] * gscale).reshape(1, 4 * D)               # [1, 1024]

    fcf1 = np.ascontiguousarray(fcf_w[:D, :])             # [256, 1]
    id64 = np.eye(64, dtype=F32)

    shared = {
        "w1_hc": w1_hc,
        "w1_enc": w1_enc,
        "b1r": b1.reshape(1, E).copy(),
        "w2_bf": np.repeat(w2, 128, axis=1).astype(BF16),  # [E, 128] replicated
        "w_hhT": w_hhT,
        "b_row": b_row,
        "w_row": w_row,
        "fcf1": fcf1,
        "id64": id64,
    }

    in_maps = []
    for ci in range(NCORES):
        sl = slice(ci * BL, (ci + 1) * BL)
        enc_l = enc[sl]                                   # [64, T, E]
        # [e, t, b] t-major free layout for the init matmul rhs
        enc_T = np.ascontiguousarray(enc_l.transpose(2, 1, 0)).reshape(E, T * BL)
        encfc = np.ascontiguousarray(enc_l @ fc_w[:E, 0:1])[:, :, 0]    # [64, T]
        encfcf = np.ascontiguousarray(enc_l @ fcf_w[D:, 0:1])[:, :, 0]  # [64, T]
        yterm = y_hist[sl] * fc_w[E, 0] + fc_b[0]         # [64, T]
        m = dict(shared)
        m.update(
            {
                "enc_T": enc_T,
                "encfc": encfc.astype(F32),
                "encfcf": encfcf.astype(F32),
                "yterm": yterm.astype(F32),
            }
        )
        in_maps.append(m)
    return in_maps, float(fcf_b[0])


def _build_program(fcf_b, n_steps=NSTEPS):
    from contextlib import ExitStack

    import concourse.bacc as bacc
    import concourse.tile as tile
    from concourse import mybir

    dt = mybir.dt
    AF = mybir.ActivationFunctionType
    OP = mybir.AluOpType

    nc = bacc.Bacc("TRN2", debug=False, num_devices=NCORES)

    # ---- DRAM I/O ------------------------------------------------------
    d_encT = nc.dram_tensor("enc_T", [E, T * BL], dt.float32, kind="ExternalInput").ap()
    d_w1hc = nc.dram_tensor("w1_hc", [2 * D, E], dt.float32, kind="ExternalInput").ap()
    d_w1enc = nc.dram_tensor("w1_enc", [E, E], dt.float32, kind="ExternalInput").ap()
    d_b1 = nc.dram_tensor("b1r", [1, E], dt.float32, kind="ExternalInput").ap()
    d_w2 = nc.dram_tensor("w2_bf", [E, 128], dt.bfloat16, kind="ExternalInput").ap()
    d_whh = nc.dram_tensor("w_hhT", [D, 4 * D], dt.float32, kind="ExternalInput").ap()
    d_brow = nc.dram_tensor("b_row", [1, 4 * D], dt.float32, kind="ExternalInput").ap()
    d_wrow = nc.dram_tensor("w_row", [1, 4 * D], dt.float32, kind="ExternalInput").ap()
    d_encfc = nc.dram_tensor("encfc", [BL, T], dt.float32, kind="ExternalInput").ap()
    d_encfcf = nc.dram_tensor("encfcf", [BL, T], dt.float32, kind="ExternalInput").ap()
    d_yterm = nc.dram_tensor("yterm", [BL, T], dt.float32, kind="ExternalInput").ap()
    d_fcf1 = nc.dram_tensor("fcf1", [D, 1], dt.float32, kind="ExternalInput").ap()
    d_id64 = nc.dram_tensor("id64", [64, 64], dt.float32, kind="ExternalInput").ap()
    d_out = nc.dram_tensor("out", [1, BL], dt.float32, kind="ExternalOutput").ap()

    FB = T * BL  # 8192 free elems per e-chunk

    with tile.TileContext(nc) as tc, ExitStack() as ctx:
        consts = ctx.enter_context(tc.tile_pool(name="consts", bufs=1))
        initp = ctx.enter_context(tc.tile_pool(name="initp", bufs=2))
        work = ctx.enter_context(tc.tile_pool(name="work", bufs=2))
        pscores = ctx.enter_context(tc.tile_pool(name="pscores", bufs=3, space="PSUM"))
        pgates = ctx.enter_context(tc.tile_pool(name="pgates", bufs=1, space="PSUM"))
        phc = ctx.enter_context(tc.tile_pool(name="phc", bufs=2, space="PSUM"))
        py = ctx.enter_context(tc.tile_pool(name="py", bufs=1, space="PSUM"))

        # ---- static SBUF ------------------------------------------------
        sb_w1hc = consts.tile([128, 4, E], dt.float32)       # k-chunks of W1_hc
        nc.sync.dma_start(sb_w1hc, d_w1hc.rearrange("(i p) e -> p i e", i=4))
        sb_w1enc = consts.tile([128, 2, E], dt.float32)
        nc.sync.dma_start(sb_w1enc, d_w1enc.rearrange("(i p) e -> p i e", i=2))
        sb_b1 = consts.tile([1, E], dt.float32)
        nc.sync.dma_start(sb_b1, d_b1)
        sb_w2 = consts.tile([128, 2, 128], dt.bfloat16)
        nc.sync.dma_start(sb_w2, d_w2.rearrange("(i p) e -> p i e", i=2))
        sb_whh = consts.tile([128, 2, 4 * D], dt.float32)
        nc.sync.dma_start(sb_whh, d_whh.rearrange("(i p) g -> p i g", i=2))
        sb_brow = consts.tile([1, 4 * D], dt.float32)
        nc.sync.dma_start(sb_brow, d_brow)
        sb_wrow = consts.tile([1, 4 * D], dt.float32)
        nc.sync.dma_start(sb_wrow, d_wrow)
        sb_encfc = consts.tile([BL, T], dt.float32)
        nc.sync.dma_start(sb_encfc, d_encfc)
        sb_encfcf = consts.tile([BL, T], dt.float32)
        nc.sync.dma_start(sb_encfcf, d_encfcf)
        sb_yterm = consts.tile([BL, T], dt.float32)
        nc.sync.dma_start(sb_yterm, d_yterm)
        sb_fcf1 = consts.tile([128, 2, 1], dt.float32)
        nc.sync.dma_start(sb_fcf1, d_fcf1.rearrange("(i p) e -> p i e", i=2))
        sb_id64 = consts.tile([64, 64], dt.float32)
        nc.sync.dma_start(sb_id64, d_id64)

        # persistent working tensors
        sb_encproj = [consts.tile([128, FB], dt.bfloat16, name=f"encproj{i}") for i in range(2)]
        sb_tval = [consts.tile([128, FB], dt.bfloat16, name=f"tval{i}") for i in range(2)]
        sb_hT = consts.tile([128, 128], dt.float32)   # [d%128, 64*blk+b]
        sb_cT = consts.tile([128, 128], dt.float32)
        nc.vector.memset(sb_hT, 0.0)
        nc.vector.memset(sb_cT, 0.0)
        sb_ones = consts.tile([1, 64], dt.float32)
        nc.vector.memset(sb_ones, 1.0)
        sb_ytT = consts.tile([1, 64], dt.float32)     # y_tilde^T, written per step

        # ---- init: enc_projT = W1_enc.T @ enc_T  (bf16 out) -------------
        CC = 512  # column chunk
        for cc in range(FB // CC):
            es0 = initp.tile([128, CC], dt.float32, name="es0")
            nc.sync.dma_start(es0, d_encT[0:128, cc * CC : (cc + 1) * CC])
            es1 = initp.tile([128, CC], dt.float32, name="es1")
            nc.sync.dma_start(es1, d_encT[128:256, cc * CC : (cc + 1) * CC])
            for ec in range(2):
                ip = pscores.tile([128, 512], dt.float32, name="ps", tag="ps")
                nc.tensor.matmul(ip, sb_w1enc[:, 0, 128 * ec : 128 * (ec + 1)], es0,
                                 start=True, stop=False)
                nc.tensor.matmul(ip, sb_w1enc[:, 1, 128 * ec : 128 * (ec + 1)], es1,
                                 start=False, stop=True)
                nc.vector.tensor_copy(sb_encproj[ec][:, cc * CC : (cc + 1) * CC], ip)

        # ---- recurrence: two independent half-batch pipelines -----------
        # Half h owns b-local [32h, 32h+32).  Chunk (h, q, j) covers
        # b = 32h + 8j + 4q (+0..4); it is computed into psum row-strip 32j
        # and drained to scc[:, 2h+q, :].  One scatter DMA per half.
        step_tiles = {}

        def emit_pre(s, h):
            bsl = slice(32 * h, 32 * h + 32)
            hcbf = []
            for ec in range(2):
                ph = phc.tile([128, 64], dt.float32, name=f"ph{h}", tag=f"ph{h}")
                esl = slice(128 * ec, 128 * (ec + 1))
                nc.tensor.matmul(ph[:, 32 * ec : 32 * ec + 32], sb_w1hc[:, 0, esl],
                                 sb_hT[:, bsl], start=True, stop=False)
                nc.tensor.matmul(ph[:, 32 * ec : 32 * ec + 32], sb_w1hc[:, 1, esl],
                                 sb_hT[:, 64 + 32 * h : 96 + 32 * h], start=False, stop=False)
                nc.tensor.matmul(ph[:, 32 * ec : 32 * ec + 32], sb_w1hc[:, 2, esl],
                                 sb_cT[:, bsl], start=False, stop=False)
                nc.tensor.matmul(ph[:, 32 * ec : 32 * ec + 32], sb_w1hc[:, 3, esl],
                                 sb_cT[:, 64 + 32 * h : 96 + 32 * h], start=False, stop=False)
                nc.tensor.matmul(ph[:, 32 * ec : 32 * ec + 32], sb_b1[:, esl],
                                 sb_ones[:, 0:32], start=False, stop=True)
            hb = work.tile([128, 64], dt.bfloat16, name=f"hcbf{h}")
            nc.vector.tensor_copy(hb, ph)
            # broadcast add: tval[ec][:, :, bsl] = encproj + hc  (t-bcast)
            for ec in range(2):
                srcv = sb_encproj[ec].rearrange("p (t b) -> p t b", b=BL)[:, :, bsl]
                dstv = sb_tval[ec].rearrange("p (t b) -> p t b", b=BL)[:, :, bsl]
                bc = hb[:, 32 * ec : 32 * ec + 32].unsqueeze(1).broadcast_to((128, T, 32))
                nc.vector.tensor_tensor(dstv, srcv, bc, op=OP.add)
            return

        def emit_tanh(s, h):
            bsl = slice(32 * h, 32 * h + 32)
            for ec in range(2):
                v = sb_tval[ec].rearrange("p (t b) -> p t b", b=BL)[:, :, bsl]
                nc.scalar.activation(v, v, AF.Tanh)

        def emit_post(s, h):
            bsl = slice(32 * h, 32 * h + 32)
            if h == 0:
                step_tiles["scores_sb"] = work.tile([BL, T], dt.float32, name="scores_sb")
                step_tiles["scc"] = work.tile([128, 4, 512], dt.float32, name="scc")
                step_tiles["exp_s"] = work.tile([BL, T], dt.float32, name="exp_s")
                step_tiles["sumexp"] = work.tile([BL, 1], dt.float32, name="sumexp")
                step_tiles["recip"] = work.tile([BL, 1], dt.float32, name="recip")
            scores_sb = step_tiles["scores_sb"]
            scc = step_tiles["scc"]
            exp_s = step_tiles["exp_s"]
            sumexp = step_tiles["sumexp"]
            recip = step_tiles["recip"]
            tv = [t.rearrange("p (t b) -> p t b", b=BL) for t in sb_tval]
            for q in range(2):
                ps = pscores.tile([128, 512], dt.float32, name="ps", tag="ps")
                for j in range(4):
                    b0 = 32 * h + 8 * j + 4 * q
                    out = ps[32 * j : 32 * (j + 1), :]
                    rhs0 = tv[0][:, :, b0 : b0 + 4].transpose([0, 2, 1])
                    rhs1 = tv[1][:, :, b0 : b0 + 4].transpose([0, 2, 1])
                    nc.tensor.matmul(out, sb_w2[:, 0, 0:32], rhs0, start=True, stop=False,
                                     tile_position=(0, 32 * j))
                    nc.tensor.matmul(out, sb_w2[:, 1, 0:32], rhs1, start=False, stop=True,
                                     tile_position=(0, 32 * j))
                nc.vector.tensor_copy(scc[:, 2 * h + q, :], ps)
            # scatter: scc[32j, 2h+q, (i t)] -> scores_sb row 32h + 8j + 4q + i
            nc.sync.dma_start(
                scores_sb[bsl, :],
                scc[0:128:32, 2 * h : 2 * h + 2, :].rearrange("p q (i t) -> p q i t", t=T),
            )

            # softmax pieces (no max-subtract: |scores| < ~5)
            nc.scalar.activation(exp_s[bsl, :], scores_sb[bsl, :], AF.Exp,
                                 accum_out=sumexp[bsl, :])
            nc.vector.reciprocal(recip[bsl, :], sumexp[bsl, :])

            # y_tilde = (sum_t exp_s*encfc) * recip + yterm[:, s]
            ttr = work.tile([32, T], dt.float32, name=f"ttr{h}")
            ydot = work.tile([32, 1], dt.float32, name=f"ydot{h}")
            nc.vector.tensor_tensor(ttr, exp_s[bsl, :], sb_encfc[bsl, :], op=OP.mult)
            nc.vector.tensor_reduce(ydot, ttr, axis=mybir.AxisListType.X, op=OP.add)
            yt = work.tile([32, 1], dt.float32, name=f"yt{h}")
            nc.vector.tensor_tensor(yt, ydot, recip[bsl, :], op=OP.mult)
            nc.vector.tensor_tensor(yt, yt, sb_yterm[bsl, s : s + 1], op=OP.add)
            pyt = py.tile([1, 32], dt.float32, name=f"pyt{h}", tag="pyt")
            nc.tensor.transpose(pyt, yt, sb_id64[0:32, 0:32])
            nc.vector.tensor_copy(sb_ytT[:, bsl], pyt)

            # gates = w_hh.T @ h + B' x ones + w_ih' x y_tildeT
            pg = pgates.tile([128, 8 * 32], dt.float32, name=f"pg{h}", tag=f"pg{h}")
            for gj in range(8):
                gsl = slice(128 * gj, 128 * (gj + 1))
                o = pg[:, 32 * gj : 32 * (gj + 1)]
                nc.tensor.matmul(o, sb_whh[:, 0, gsl], sb_hT[:, bsl], start=True, stop=False)
                nc.tensor.matmul(o, sb_whh[:, 1, gsl], sb_hT[:, 64 + 32 * h : 96 + 32 * h],
                                 start=False, stop=False)
                nc.tensor.matmul(o, sb_brow[:, gsl], sb_ones[:, 0:32], start=False, stop=False)
                nc.tensor.matmul(o, sb_wrow[:, gsl], sb_ytT[:, bsl], start=False, stop=True)

            # Tg = tanh(0.5 * gates): blocks [i0 i1 f0 f1 g0 g1 o0 o1] x 32
            T_sb = work.tile([128, 256], dt.float32, name=f"T_sb{h}")
            nc.scalar.activation(T_sb, pg, AF.Tanh, scale=0.5)
            Tv = T_sb.rearrange("p (g b) -> p g b", b=32)
            Ti, Tf, Tg, To = (Tv[:, 2 * k : 2 * k + 2, :] for k in range(4))
            cv = sb_cT.rearrange("p (k b) -> p k b", b=32)[:, h::2, :]
            hv = sb_hT.rearrange("p (k b) -> p k b", b=32)[:, h::2, :]

            # c' = ((Tf+1)*c + (Ti+1)*Tg)/2 ; h' = (To+1)*tanh(c')/2
            tmp1 = work.tile([128, 64], dt.float32, name=f"tmp1{h}")
            tmp2 = work.tile([128, 64], dt.float32, name=f"tmp2{h}")
            tmp3 = work.tile([128, 64], dt.float32, name=f"tmp3{h}")
            t1v = tmp1.rearrange("p (k b) -> p k b", b=32)
            t2v = tmp2.rearrange("p (k b) -> p k b", b=32)
            t3v = tmp3.rearrange("p (k b) -> p k b", b=32)
            nc.vector.tensor_tensor(t1v, Tf, cv, op=OP.mult)
            nc.vector.tensor_add(t1v, t1v, cv)
            nc.vector.tensor_tensor(t2v, Ti, Tg, op=OP.mult)
            nc.vector.tensor_add(t2v, t2v, Tg)
            nc.vector.tensor_add(t1v, t1v, t2v)          # 2*c_new
            nc.vector.tensor_scalar_mul(cv, t1v, 0.5)
            nc.scalar.activation(t2v, t1v, AF.Tanh, scale=0.5)  # tanh(c_new)
            nc.vector.tensor_tensor(t3v, To, t2v, op=OP.mult)
            nc.vector.tensor_add(t3v, t3v, t2v)
            nc.vector.tensor_scalar_mul(hv, t3v, 0.5)

        for s in range(n_steps):
            emit_pre(s, 0)
            emit_pre(s, 1)
            emit_tanh(s, 0)
            emit_tanh(s, 1)
            emit_post(s, 0)
            emit_post(s, 1)
        exp_s = step_tiles.get("exp_s")
        recip = step_tiles.get("recip")

        # ---- final output ----------------------------------------------
        _emit_final(nc, tc, work, py, dt, AF, OP, exp_s, recip, sb_encfcf,
                    sb_fcf1, sb_hT, sb_id64, d_out, fcf_b)

    nc.compile()
    return nc


def _emit_final(nc, tc, work, py, dt, AF, OP, exp_s, recip, sb_encfcf,
                sb_fcf1, sb_hT, sb_id64, d_out, fcf_b):
        ttrf = work.tile([BL, T], dt.float32, name="ttrf")
        fdot = work.tile([BL, 1], dt.float32, name="fdot")
        from concourse import mybir as _mb
        nc.vector.tensor_tensor(ttrf, exp_s, sb_encfcf, op=OP.mult)
        nc.vector.tensor_reduce(fdot, ttrf, axis=_mb.AxisListType.X, op=OP.add)
        nc.vector.tensor_tensor(fdot, fdot, recip, op=OP.mult)
        f2T = work.tile([1, 64], dt.float32, name="f2T")
        if "nopetr" in DBG:
            nc.sync.dma_start(f2T, fdot)
        else:
            pf = py.tile([1, 64], dt.float32, name="pyt", tag="pyt")
            nc.tensor.transpose(pf, fdot, sb_id64)
            nc.vector.tensor_copy(f2T, pf)

        pfin = py.tile([1, 64], dt.float32, name="pyt", tag="pyt")
        nc.tensor.matmul(pfin, sb_fcf1[:, 0, :], sb_hT[:, 0:64], start=True, stop=False)
        nc.tensor.matmul(pfin, sb_fcf1[:, 1, :], sb_hT[:, 64:128], start=False, stop=True)
        out_sb = work.tile([1, 64], dt.float32, name="out_sb")
        nc.vector.tensor_tensor(out_sb, pfin, f2T, op=OP.add)
        nc.vector.tensor_scalar_add(out_sb, out_sb, fcf_b)
        nc.sync.dma_start(d_out, out_sb)


def kernel(**inputs):
    global _LAST_RESULTS, _LAST_WALL_NS
    import time

    from concourse.bass_utils import run_bass_kernel_spmd

    in_maps, fcf_b = _host_prepare(inputs)
    nc = _build_program(fcf_b)
    t0 = time.time()
    res = run_bass_kernel_spmd(nc, in_maps, core_ids=list(range(NCORES)))
    _LAST_WALL_NS = (time.time() - t0) * 1e9
    _LAST_RESULTS = res
    out = np.concatenate([r["out"].reshape(BL, OUT) for r in res.results], axis=0)
    return out.astype(np.float32)


if __name__ == "__main__":
    rng = np.random.default_rng(0)
    fake = {
        "input_encoded": rng.standard_normal((B, T, E), dtype=np.float32),
        "y_history": rng.standard_normal((B, T), dtype=np.float32),
        "attn_w1": 0.05 * rng.standard_normal((2 * D + E, E), dtype=np.float32),
        "attn_b1": 0.05 * rng.standard_normal((E,), dtype=np.float32),
        "attn_w2": 0.05 * rng.standard_normal((E, 1), dtype=np.float32),
        "attn_b2": 0.05 * rng.standard_normal((1,), dtype=np.float32),
        "lstm_w_ih": 0.05 * rng.standard_normal((4 * D, OUT), dtype=np.float32),
        "lstm_w_hh": 0.05 * rng.standard_normal((4 * D, D), dtype=np.float32),
        "lstm_b_ih": 0.05 * rng.standard_normal((4 * D,), dtype=np.float32),
        "lstm_b_hh": 0.05 * rng.standard_normal((4 * D,), dtype=np.float32),
        "fc_w": rng.standard_normal((E + OUT, OUT), dtype=np.float32),
        "fc_b": 0.05 * rng.standard_normal((OUT,), dtype=np.float32),
        "fcf_w": 0.05 * rng.standard_normal((D + E, OUT), dtype=np.float32),
        "fcf_b": 0.05 * rng.standard_normal((OUT,), dtype=np.float32),
    }
    out = kernel(**fake)
    print("kernel out", out.shape, out[:4, 0])
